# revision 1
# baseline (speedup 1.0000x reference)
"""DDiT block kernel for 8 Trainium2 NeuronCores.

Sharding: data-parallel over (batch, sequence-half) -> 8 shards. Each core
processes one batch's full sequence through LN1/K/V (needed for attention),
but only its 512 query tokens through Q/attention/MLP.

Device layout is feature-major (model dim on partitions, tokens on the free
axis), which makes every adaLN modulation a per-partition scalar and lets all
matmuls consume activations without transposes. Host folds the adaLN scale
and LN weight into the qkv/mlp1 weights, and the shift-vectors into biases.
Tokens are rotated per-core so queries are always tokens [0:512).

Matmuls run in bf16 with fp32 PSUM accumulation; LN stats, softmax and
residuals stay fp32.
"""

import numpy as np
import ml_dtypes

BF = ml_dtypes.bfloat16

B, S, D, H, HD = 4, 1024, 1024, 16, 64
Q = 512          # queries per core
KO = 8           # 1024 dim / 128 partitions
MLP = 4096
LN_EPS = 1e-5

_CACHE = {}


# ----------------------------------------------------------------------------
# host-side layout helpers
# ----------------------------------------------------------------------------

def _pieces(W, m_piece):
    """[K, M] weight -> [n_pieces, 128, K//128, m_piece] bf16, contiguous."""
    K, M = W.shape
    ko = K // 128
    Wr = np.asarray(W, np.float32).reshape(ko, 128, M).transpose(1, 0, 2)
    n = M // m_piece
    out = Wr.reshape(128, ko, n, m_piece).transpose(2, 0, 1, 3)
    return np.ascontiguousarray(out.astype(BF))


def _pvec(v):
    """[M] vector -> [128, M//128] f32 (partition-major chunks)."""
    v = np.asarray(v, np.float32)
    return np.ascontiguousarray(v.reshape(-1, 128).T)


# ----------------------------------------------------------------------------
# device program
# ----------------------------------------------------------------------------

def _build_program(repeat=1):
    import concourse.bass as bass
    import concourse.mybir as mybir
    import concourse.tile as tile
    from concourse import bacc

    f32 = mybir.dt.float32
    bf = mybir.dt.bfloat16
    AF = mybir.ActivationFunctionType
    ALU = mybir.AluOpType
    ts = bass.ts

    nc = bacc.Bacc("TRN2", target_bir_lowering=False, debug=False,
                   enable_asserts=False)

    def din(name, shape, dt=bf):
        return nc.dram_tensor(name, shape, dt, kind="ExternalInput").ap()

    xb_d = din("xb", [D, S])                      # bf16 x, feature-major
    xs_d = din("xskip", [D, Q], f32)              # f32 skip, feature-major
    wq_d = din("wq", [2, 128, KO, 512])
    wk_d = din("wk", [2, 128, KO, 512])
    wv_d = din("wv", [2, 128, KO, 512])
    wo_d = din("wao", [2, 128, KO, 512])
    w1_d = din("wm1", [8, 128, KO, 512])
    w2_d = din("wm2", [8, 128, 32, 128])
    cc_d = din("cc", [128, S])
    ss_d = din("ss", [128, S])
    bq_d = din("bq", [128, KO], f32)
    bk_d = din("bk", [128, KO], f32)
    bo_d = din("bao", [128, KO], f32)
    gm_d = din("gmsa", [128, KO], f32)
    b1_d = din("bm1", [128, 32], f32)
    b2_d = din("bm2", [128, KO], f32)
    gp_d = din("gmlp", [128, KO], f32)
    yt_d = nc.dram_tensor("yt", [D, Q], f32, kind="ExternalOutput").ap()

    with tile.TileContext(nc) as tc:
        with tc.tile_pool(name="sb", bufs=1) as sb, \
             tc.tile_pool(name="ps", bufs=1, space="PSUM") as ps:
            for _rep in range(repeat):

                def psum():
                    return ps.tile([128, 512], f32, tag="p", bufs=4, name="pt")

                def psum2():
                    return ps.tile([128, 1024], f32, tag="p2", bufs=2, name="pt2")

                def tmpf():
                    return sb.tile([128, 512], f32, tag="tmpf", bufs=4, name="tf")

                # ---- P0: input DMAs first (xb chunks feed LN1 asap) ----
                xb8 = []
                for ko in range(KO):
                    t = sb.tile([128, S], bf, tag="xb8", bufs=8, name="xb")
                    nc.sync.dma_start(
                        t[:], xb_d.rearrange("(ko p) t -> p ko t", p=128)[:, ko, :])
                    xb8.append(t)
                xskip = sb.tile([128, KO, Q], f32, tag="xskip", bufs=1)
                nc.sync.dma_start(xskip[:], xs_d.rearrange("(ko p) t -> p ko t", p=128))
                ones_b = sb.tile([128, 128], bf, tag="ones", bufs=2)
                nc.vector.memset(ones_b[:], 1.0)
                eps_ap = sb.tile([128, 1], f32, tag="eps", bufs=1)
                nc.vector.memset(eps_ap[:], LN_EPS)
                csb = sb.tile([128, S], bf, tag="cs", bufs=2)
                nc.sync.dma_start(csb[:], cc_d[:])
                ssb = sb.tile([128, S], bf, tag="cs", bufs=2)
                nc.sync.dma_start(ssb[:], ss_d[:])

                def small(dram, n):
                    t = sb.tile([128, n], f32, tag="bias", bufs=7, name="bias")
                    nc.sync.dma_start(t[:], dram[:])
                    return t

                bq_s, bk_s, bo_s, gm_s = (small(x, KO) for x in (bq_d, bk_d, bo_d, gm_d))
                b1_s = small(b1_d, 32)
                b2_s, gp_s = small(b2_d, KO), small(gp_d, KO)

                def wpiece(dram, i, shape, tag="w8", bufs=6):
                    t = sb.tile(shape, bf, tag=tag, bufs=bufs, name="w")
                    nc.sync.dma_start(t[:], dram[i])
                    return t

                wq_sb = [wpiece(wq_d, i, [128, KO, 512]) for i in range(2)]
                wk_sb = [wpiece(wk_d, i, [128, KO, 512]) for i in range(2)]
                wv_sb = [wpiece(wv_d, i, [128, KO, 512]) for i in range(2)]

                # ---- P1: LN1 over all 1024 tokens ----
                ps_s1 = [psum() for _ in range(2)]
                ps_s2 = [psum() for _ in range(2)]
                for ko in range(KO):
                    sqk = sb.tile([128, S], bf, tag="kslab", bufs=4, name="sqk")
                    nc.scalar.square(sqk[:], xb8[ko][:])
                    for tb in range(2):
                        nc.tensor.matmul(ps_s1[tb][:], ones_b[:], xb8[ko][:, ts(tb, 512)],
                                         start=(ko == 0), stop=(ko == KO - 1))
                        nc.tensor.matmul(ps_s2[tb][:], ones_b[:], sqk[:, ts(tb, 512)],
                                         start=(ko == 0), stop=(ko == KO - 1))

                mu01 = sb.tile([128, 1024], bf, tag="stats16", bufs=4, name="mu01")
                rstd01 = sb.tile([128, 1024], bf, tag="stats16", bufs=4, name="rstd01")
                for tb in range(2):
                    mu = tmpf()
                    nc.vector.tensor_scalar_mul(mu[:], ps_s1[tb][:], 1.0 / D)
                    ex2 = tmpf()
                    nc.vector.tensor_scalar_mul(ex2[:], ps_s2[tb][:], 1.0 / D)
                    var = tmpf()
                    nc.vector.tensor_tensor(var[:], mu[:], mu[:], ALU.mult)
                    nc.vector.tensor_tensor(var[:], ex2[:], var[:], ALU.subtract)
                    sd = tmpf()
                    nc.scalar.activation(sd[:], var[:], AF.Sqrt, bias=eps_ap[:])
                    nc.vector.tensor_copy(mu01[:, ts(tb, 512)], mu[:])
                    with nc.allow_low_precision(reason="bf16 LN rstd"):
                        nc.vector.reciprocal(rstd01[:, ts(tb, 512)], sd[:])

                g16 = []
                for ko in range(KO):
                    g = sb.tile([128, S], bf, tag="act2k", bufs=16, name="g16")
                    tm = sb.tile([128, S], bf, tag="kslab", bufs=4, name="tm")
                    nc.vector.tensor_tensor(tm[:], xb8[ko][:], mu01[:], ALU.subtract)
                    nc.vector.tensor_tensor(g[:], tm[:], rstd01[:], ALU.mult)
                    g16.append(g)

                # ---- P2: projections + rope ----
                def swap_dma(dst, src, n):
                    # swap 32-row halves within each 64-row (head) block
                    for g in range(2):
                        r = g * 64
                        nc.sync.dma_start(dst[r:r + 32, :n], src[r + 32:r + 64, :n])
                        nc.sync.dma_start(dst[r + 32:r + 64, :n], src[r:r + 32, :n])

                qr8 = []
                for jo in range(KO):
                    pq = psum()
                    for ko in range(KO):
                        nc.tensor.matmul(pq[:], wq_sb[jo // 4][:, ko, ts(jo % 4, 128)],
                                         g16[ko][:, 0:Q], start=(ko == 0), stop=(ko == KO - 1))
                    qa = sb.tile([128, 512], bf, tag="qslab", bufs=6, name="qa")
                    nc.scalar.add(qa[:, :Q], pq[:, :Q], bq_s[:, jo:jo + 1])
                    qsw = sb.tile([128, 512], bf, tag="qslab", bufs=6, name="qsw")
                    swap_dma(qsw, qa, Q)
                    t1 = sb.tile([128, 512], bf, tag="qslab", bufs=6, name="qt1")
                    nc.vector.tensor_tensor(t1[:, :Q], qa[:, :Q], csb[:, 0:Q], ALU.mult)
                    t2 = sb.tile([128, 512], bf, tag="qslab", bufs=6, name="qt2")
                    nc.vector.tensor_tensor(t2[:, :Q], qsw[:, :Q], ssb[:, 0:Q], ALU.mult)
                    qr = sb.tile([128, Q], bf, tag="act1k", bufs=16, name="qr")
                    nc.vector.tensor_tensor(qr[:], t1[:, :Q], t2[:, :Q], ALU.add)
                    qr8.append(qr)

                kr8 = []
                for jo in range(KO):
                    ka = sb.tile([128, S], bf, tag="kslab", bufs=4, name="ka")
                    for tb in range(2):
                        pk = psum()
                        for ko in range(KO):
                            nc.tensor.matmul(pk[:], wk_sb[jo // 4][:, ko, ts(jo % 4, 128)],
                                             g16[ko][:, ts(tb, 512)],
                                             start=(ko == 0), stop=(ko == KO - 1))
                        nc.scalar.add(ka[:, ts(tb, 512)], pk[:], bk_s[:, jo:jo + 1])
                    ksw = sb.tile([128, S], bf, tag="kslab", bufs=4, name="ksw")
                    swap_dma(ksw, ka, S)
                    t1 = sb.tile([128, S], bf, tag="kslab", bufs=4, name="kt1")
                    nc.vector.tensor_tensor(t1[:], ka[:], csb[:], ALU.mult)
                    nc.vector.tensor_tensor(ksw[:], ksw[:], ssb[:], ALU.mult)
                    kr = sb.tile([128, S], bf, tag="act2k", bufs=16, name="kr")
                    nc.vector.tensor_tensor(kr[:], t1[:], ksw[:], ALU.add)
                    kr8.append(kr)

                # v, token-major, with a ones-column per head (denominator trick)
                v_sb = sb.tile([128, KO, H, 66], bf, tag="m16v", bufs=1, name="vsb")
                nc.vector.memset(v_sb[:, :, :, 64:65], 1.0)
                for to in range(KO):
                    for nb in range(2):
                        pv = psum()
                        for ko in range(KO):
                            nc.tensor.matmul(pv[:], g16[ko][:, ts(to, 128)],
                                             wv_sb[nb][:, ko, :],
                                             start=(ko == 0), stop=(ko == KO - 1))
                        nc.scalar.copy(v_sb[:, to, nb * 8:(nb + 1) * 8, 0:64],
                                       pv[:].rearrange("p (h d) -> p h d", d=64))

                # ---- P3: attention (scoresT layout), head pairs interleaved ----
                wo_sb = [wpiece(wo_d, i, [128, KO, 512]) for i in range(2)]
                oT8 = [sb.tile([128, Q], bf, tag="act1k", bufs=16, name="oT")
                       for _ in range(KO)]
                for hp in range(8):
                    jo = hp
                    probs = {0: [], 1: []}
                    for half in range(4):          # 2 key-tiles per chunk
                        pbig = {}
                        for sub in range(2):       # the 2 heads of the pair
                            r0 = sub * 64
                            big = psum2()
                            for kk in range(2):
                                kt = half * 2 + kk
                                nc.tensor.matmul(big[:, ts(kk, 512)],
                                                 kr8[jo][r0:r0 + 64, ts(kt, 128)],
                                                 qr8[jo][r0:r0 + 64, :],
                                                 start=True, stop=True,
                                                 tile_position=(r0, 0))
                            pbig[sub] = big
                        for sub in range(2):
                            pb = sb.tile([128, 1024], bf, tag="probs", bufs=3, name="pb")
                            nc.scalar.activation(pb[:], pbig[sub][:], AF.Exp, scale=0.125)
                            probs[sub].append(pb)
                    po2 = {}
                    for sub in range(2):
                        h = 2 * hp + sub
                        po = psum()
                        for kt in range(KO):
                            nc.tensor.matmul(po[0:65, :], v_sb[:, kt, h, 0:65],
                                             probs[sub][kt // 2][:, ts(kt % 2, 512)],
                                             start=(kt == 0), stop=(kt == KO - 1))
                        po2[sub] = po
                    for sub in range(2):
                        h = 2 * hp + sub
                        r0 = sub * 64
                        po = po2[sub]
                        rcp = sb.tile([65, 512], bf, tag="rcp", bufs=2, name="rcp")
                        with nc.allow_low_precision(reason="bf16 softmax denominator"):
                            nc.vector.reciprocal(rcp[64:65, :], po[64:65, :])
                        prb = psum()
                        nc.tensor.matmul(prb[0:64, :], ones_b[64:65, 0:64], rcp[64:65, :],
                                         start=True, stop=True)
                        rb = sb.tile([64, 512], bf, tag="rb", bufs=2, name="rb")
                        nc.vector.tensor_copy(rb[:], prb[0:64, :])
                        o16 = sb.tile([64, 512], bf, tag="o16", bufs=2, name="o16")
                        nc.vector.tensor_tensor(o16[:], po[0:64, :], rb[:], ALU.mult)
                        nc.sync.dma_start(oT8[jo][r0:r0 + 64, :], o16[:])

                # ---- P4: attn out + gated residual ----
                x2 = []
                for do in range(KO):
                    py = psum()
                    for ko in range(KO):
                        nc.tensor.matmul(py[:], wo_sb[do // 4][:, ko, ts(do % 4, 128)],
                                         oT8[ko][:], start=(ko == 0), stop=(ko == KO - 1))
                    t = tmpf()
                    nc.scalar.activation(t[:], py[:], AF.Identity,
                                         bias=bo_s[:, do:do + 1],
                                         scale=gm_s[:, do:do + 1])
                    xx = sb.tile([128, Q], f32, tag="act2k", bufs=16, name="x2")
                    nc.vector.tensor_tensor(xx[:], t[:], xskip[:, do], ALU.add)
                    x2.append(xx)

                # ---- P5: LN2 (512 tokens) ----
                p1 = psum()
                p2 = psum()
                x2b = []
                for ko in range(KO):
                    xc = sb.tile([128, Q], bf, tag="act1k", bufs=16, name="x2b")
                    nc.scalar.copy(xc[:], x2[ko][:])
                    x2b.append(xc)
                    sq2 = sb.tile([128, S], bf, tag="kslab", bufs=4, name="sq2")
                    nc.scalar.square(sq2[:, 0:Q], x2[ko][:])
                    nc.tensor.matmul(p1[:], ones_b[:], xc[:], start=(ko == 0),
                                     stop=(ko == KO - 1))
                    nc.tensor.matmul(p2[:], ones_b[:], sq2[:, 0:Q], start=(ko == 0),
                                     stop=(ko == KO - 1))
                mu = tmpf()
                nc.vector.tensor_scalar_mul(mu[:], p1[:], 1.0 / D)
                ex2 = tmpf()
                nc.vector.tensor_scalar_mul(ex2[:], p2[:], 1.0 / D)
                var = tmpf()
                nc.vector.tensor_tensor(var[:], mu[:], mu[:], ALU.mult)
                nc.vector.tensor_tensor(var[:], ex2[:], var[:], ALU.subtract)
                sd = tmpf()
                nc.scalar.activation(sd[:], var[:], AF.Sqrt, bias=eps_ap[:])
                mu16 = sb.tile([128, 512], bf, tag="stats16", bufs=4, name="mu16b")
                nc.vector.tensor_copy(mu16[:], mu[:])
                rstd16 = sb.tile([128, 512], bf, tag="stats16", bufs=4, name="rstd16b")
                with nc.allow_low_precision(reason="bf16 LN rstd"):
                    nc.vector.reciprocal(rstd16[:], sd[:])
                g2 = []
                for ko in range(KO):
                    tm = sb.tile([128, 512], bf, tag="qslab", bufs=6, name="tm2")
                    nc.vector.tensor_tensor(tm[:], x2b[ko][:], mu16[:], ALU.subtract)
                    gk = sb.tile([128, Q], bf, tag="act1k", bufs=16, name="g2")
                    nc.vector.tensor_tensor(gk[:], tm[:], rstd16[:], ALU.mult)
                    g2.append(gk)

                # ---- P6/P7: MLP ----
                w1_sb = [wpiece(w1_d, i, [128, KO, 512]) for i in range(8)]
                w2_sb = [wpiece(w2_d, i, [128, 32, 128]) for i in range(8)]
                m16 = sb.tile([128, 32, Q], bf, tag="m16v", bufs=1, name="m16")
                for mo in range(32):
                    pm = psum()
                    for ko in range(KO):
                        nc.tensor.matmul(pm[:], w1_sb[mo // 4][:, ko, ts(mo % 4, 128)],
                                         g2[ko][:], start=(ko == 0),
                                         stop=(ko == KO - 1))
                    nc.scalar.activation(m16[:, mo], pm[:], AF.Gelu_apprx_tanh,
                                         bias=b1_s[:, mo:mo + 1], scale=1.0)
                yt_r = yt_d.rearrange("(ko p) t -> p ko t", p=128)
                for do in range(KO):
                    pz = psum()
                    for ko in range(32):
                        nc.tensor.matmul(pz[:], w2_sb[do][:, ko, :],
                                         m16[:, ko, :], start=(ko == 0), stop=(ko == 31))
                    t = tmpf()
                    nc.scalar.activation(t[:], pz[:], AF.Identity,
                                         bias=b2_s[:, do:do + 1],
                                         scale=gp_s[:, do:do + 1])
                    nc.vector.tensor_tensor(x2[do][:], t[:], x2[do][:], ALU.add)
                    nc.sync.dma_start(yt_r[:, do, :], x2[do][:])

    nc.compile()
    return nc


# ----------------------------------------------------------------------------
# host wrapper
# ----------------------------------------------------------------------------

def _prep_shared(inputs):
    x = np.asarray(inputs["x"], np.float32)
    c = np.asarray(inputs["c"], np.float32)
    w_ada = np.asarray(inputs["w_ada"], np.float32)
    b_ada = np.asarray(inputs["b_ada"], np.float32)
    w_qkv = np.asarray(inputs["w_qkv"], np.float32)
    w_ao = np.asarray(inputs["w_attn_out"], np.float32)
    w_m1 = np.asarray(inputs["w_mlp1"], np.float32)
    w_m2 = np.asarray(inputs["w_mlp2"], np.float32)

    mod = c @ w_ada + b_ada
    sh_msa, sc_msa, g_msa, sh_mlp, sc_mlp, g_mlp = np.split(mod, 6, axis=1)
    ln1 = np.asarray(inputs["w_ln1"], np.float32) * (1.0 + sc_msa)   # [4, D]
    ln2 = np.asarray(inputs["w_ln2"], np.float32) * (1.0 + sc_mlp)

    shared = {}
    for b in range(B):
        Wq = w_qkv[:, :D] * ln1[b][:, None]
        Wk = w_qkv[:, D:2 * D] * ln1[b][:, None]
        Wv = w_qkv[:, 2 * D:] * ln1[b][:, None]
        bqkv = sh_msa[b] @ w_qkv
        W1 = w_m1 * ln2[b][:, None]
        bm1 = sh_mlp[b] @ w_m1 + np.asarray(inputs["b_mlp1"], np.float32)
        shared[b] = dict(
            wq=_pieces(Wq, 512), wk=_pieces(Wk, 512), wv=_pieces(Wv, 512),
            wm1=_pieces(W1, 512),
            bq=_pvec(bqkv[:D]), bk=_pvec(bqkv[D:2 * D]),
            bao=_pvec((bqkv[2 * D:] @ w_ao) * g_msa[b]),
            gmsa=_pvec(g_msa[b]), bm1=_pvec(bm1),
            gmlp=_pvec(g_mlp[b]),
            bm2g=_pvec(np.asarray(inputs["b_mlp2"], np.float32) * g_mlp[b]),
        )
    wao_p = _pieces(w_ao, 512)
    wm2_p = _pieces(w_m2, 128)
    bm2_p = _pvec(np.asarray(inputs["b_mlp2"], np.float32))
    cos = np.asarray(inputs["cos"], np.float32)
    sin = np.asarray(inputs["sin"], np.float32)
    return shared, wao_p, wm2_p, bm2_p, x, cos, sin


def _make_in_maps(inputs):
    shared, wao_p, wm2_p, bm2_p, x, cos, sin = _prep_shared(inputs)
    in_maps = []
    for core in range(8):
        b, half = core // 2, core % 2
        qlo = half * Q
        order = np.concatenate([np.arange(qlo, qlo + Q), np.arange(0, qlo),
                                np.arange(qlo + Q, S)])
        xT = x[b][order].T
        cosT = cos[order].T                      # [32, S]
        sinT = sin[order].T
        cc = np.concatenate([cosT] * 4, 0).astype(BF)
        ss = np.concatenate([-sinT, sinT, -sinT, sinT], 0).astype(BF)
        sh = shared[b]
        in_maps.append({
            "xb": np.ascontiguousarray(xT.astype(BF)),
            "xskip": np.ascontiguousarray(xT[:, :Q].astype(np.float32)),
            "wq": sh["wq"], "wk": sh["wk"], "wv": sh["wv"],
            "wao": wao_p, "wm1": sh["wm1"], "wm2": wm2_p,
            "cc": np.ascontiguousarray(cc), "ss": np.ascontiguousarray(ss),
            "bq": sh["bq"], "bk": sh["bk"], "bao": sh["bao"],
            "gmsa": sh["gmsa"], "bm1": sh["bm1"], "bm2": sh["bm2g"],
            "gmlp": sh["gmlp"],
        })
    return in_maps


def kernel(**inputs):
    from concourse import bass_utils

    if "nc" not in _CACHE:
        _CACHE["nc"] = _build_program()
    nc = _CACHE["nc"]

    in_maps = _make_in_maps(inputs)
    res = bass_utils.run_bass_kernel_spmd(nc, in_maps, core_ids=list(range(8)))

    y = np.zeros((B, S, D), np.float32)
    for core in range(8):
        b, half = core // 2, core % 2
        qlo = half * Q
        y[b, qlo:qlo + Q] = res.results[core]["yt"].T
    return y



# revision 2
# speedup vs baseline: 1.1482x; 1.1482x over previous
"""DDiT block kernel for 8 Trainium2 NeuronCores.

Sharding: data-parallel over (batch, sequence-half) -> 8 shards. Each core
processes one batch's full sequence through LN1/K/V (needed for attention),
but only its 512 query tokens through Q/attention/MLP.

Device layout is feature-major (model dim on partitions, tokens on the free
axis), which makes every adaLN modulation a per-partition scalar and lets all
matmuls consume activations without transposes. Host folds the adaLN scale
and LN weight into the qkv/mlp1 weights, and the shift-vectors into biases.
Tokens are rotated per-core so queries are always tokens [0:512).

QKV / attn-out projections run in fp8e4m3 DoubleRow (weights pre-scaled by
SW, compensated on PSUM read-out); scores, probs, attnV and the MLP stay
bf16 (fp8 there breaks the 2e-2 gate). LN stats, softmax and residuals are
fp32. The attention region is a per-head-pair software pipeline so PE
(matmuls), Act (exp) and DVE (rope, bias, normalize) overlap.
"""

import numpy as np
import ml_dtypes

BF = ml_dtypes.bfloat16
F8 = ml_dtypes.float8_e4m3

B, S, D, H, HD = 4, 1024, 1024, 16, 64
Q = 512          # queries per core
KO = 8           # 1024 dim / 128 partitions
MLP = 4096
LN_EPS = 1e-5
SW = 64.0        # fp8 weight pre-scale (compensated on PSUM read-out)

_CACHE = {}

# bisect flags (compile-crash isolation)
USE_TS = True         # DVE tensor_scalar for q/k bias (else Act activation)
USE_PAIR_DEN = False  # dead: partition ranges must start at 0/32/64/96


# ----------------------------------------------------------------------------
# host-side layout helpers
# ----------------------------------------------------------------------------

def _pieces(W, m_piece, dt=BF, scale=1.0):
    """[K, M] weight -> [n_pieces, 128, K//128, m_piece], contiguous."""
    K, M = W.shape
    ko = K // 128
    Wr = (np.asarray(W, np.float32) * scale).reshape(ko, 128, M).transpose(1, 0, 2)
    n = M // m_piece
    out = Wr.reshape(128, ko, n, m_piece).transpose(2, 0, 1, 3)
    return np.ascontiguousarray(out.astype(dt))


def _pvec(v):
    """[M] vector -> [128, M//128] f32 (partition-major chunks)."""
    v = np.asarray(v, np.float32)
    return np.ascontiguousarray(v.reshape(-1, 128).T)


# ----------------------------------------------------------------------------
# device program
# ----------------------------------------------------------------------------

def _build_program(repeat=1):
    import concourse.bass as bass
    import concourse.mybir as mybir
    import concourse.tile as tile
    from concourse import bacc

    f32 = mybir.dt.float32
    bf = mybir.dt.bfloat16
    f8 = mybir.dt.float8e4
    AF = mybir.ActivationFunctionType
    ALU = mybir.AluOpType
    DR = mybir.MatmulPerfMode.DoubleRow
    ts = bass.ts

    nc = bacc.Bacc("TRN2", target_bir_lowering=False, debug=False,
                   enable_asserts=False)

    def din(name, shape, dt=bf):
        return nc.dram_tensor(name, shape, dt, kind="ExternalInput").ap()

    xb_d = din("xb", [D, S])                      # bf16 x, feature-major
    wq_d = din("wq", [2, 128, KO, 512], f8)
    wk_d = din("wk", [2, 128, KO, 512], f8)
    wv_d = din("wv", [2, 128, KO, 512], f8)
    wo_d = din("wao", [2, 128, KO, 512], f8)
    w1_d = din("wm1", [8, 128, KO, 512])
    w2_d = din("wm2", [8, 128, 32, 128])
    cc_d = din("cc", [128, S])
    ss_d = din("ss", [128, S])
    bv_d = din("bvec", [128, 80], f32)
    yt_d = nc.dram_tensor("yt", [D, Q], f32, kind="ExternalOutput").ap()

    with tile.TileContext(nc) as tc:
        with tc.tile_pool(name="sb", bufs=1) as sb, \
             tc.tile_pool(name="ps", bufs=1, space="PSUM") as ps:
            for _rep in range(repeat):

                def psum():
                    return ps.tile([128, 512], f32, tag="p", bufs=2, name="pt")

                def psumv():
                    return ps.tile([128, 512], f32, tag="pv", bufs=2, name="pvt")

                def psum2():
                    return ps.tile([128, 1024], f32, tag="p2", bufs=2, name="pt2")

                def tmpf():
                    return sb.tile([128, 512], f32, tag="tmpf", bufs=3, name="tf")

                # ---- P0: DMA schedule. xb token-half tb0 first (alternating
                # SP/Act issue queues) so LN1-tb0 unblocks projections while
                # tb1 and the relaxed weights stream in behind. ----
                xb_r = xb_d.rearrange("(ko p) t -> p ko t", p=128)
                xb8 = [sb.tile([128, S], bf, tag="xb8", bufs=8, name="xb")
                       for _ in range(KO)]
                for ko in range(KO):
                    nc.sync.dma_start(xb8[ko][:, 0:512], xb_r[:, ko, 0:512])

                def wpiece(dram, i, shape, tag="w8", bufs=6, dt=bf, eng=None,
                           nsplit=2):
                    # split along the ko axis so DMA engines share a piece
                    eng = eng or nc.sync
                    t = sb.tile(shape, dt, tag=tag, bufs=bufs, name="w")
                    h = shape[1] // nsplit
                    for j in range(nsplit):
                        eng.dma_start(t[:, j * h:(j + 1) * h],
                                      dram[i][:, j * h:(j + 1) * h])
                    return t

                # earliest-needed weights as quarter transfers
                wq_sb = [None, None]
                wk_sb = [None, None]
                wv_sb = [None, None]
                wq_sb[0] = wpiece(wq_d, 0, [128, KO, 512], dt=f8, nsplit=4)
                csb = sb.tile([128, S], bf, tag="cs", bufs=2)
                ssb = sb.tile([128, S], bf, tag="cs", bufs=2)
                for j in range(2):
                    nc.sync.dma_start(csb[:, ts(j, 512)], cc_d[:, ts(j, 512)])
                    nc.sync.dma_start(ssb[:, ts(j, 512)], ss_d[:, ts(j, 512)])
                for ko in range(KO):
                    nc.sync.dma_start(xb8[ko][:, 512:1024], xb_r[:, ko, 512:1024])
                wk_sb[0] = wpiece(wk_d, 0, [128, KO, 512], dt=f8, nsplit=4)
                wv_sb[0] = wpiece(wv_d, 0, [128, KO, 512], dt=f8, nsplit=4)
                bvec = sb.tile([128, 80], f32, tag="bias", bufs=1, name="bvec")
                nc.sync.dma_start(bvec[:], bv_d[:])
                bq_s, bk_s, bo_s, gm_s = (bvec[:, 8 * i:8 * i + 8]
                                          for i in range(4))
                b1_s = bvec[:, 32:64]
                b2_s, gp_s = bvec[:, 64:72], bvec[:, 72:80]
                wq_sb[1] = wpiece(wq_d, 1, [128, KO, 512], dt=f8)
                wk_sb[1] = wpiece(wk_d, 1, [128, KO, 512], dt=f8)
                wv_sb[1] = wpiece(wv_d, 1, [128, KO, 512], dt=f8)
                # first two MLP1 weight pieces prefetch into dedicated buffers
                # (the w8 ring + SP queue only frees up after the pairs)
                w1_early = [wpiece(w1_d, i, [128, KO, 512], tag="w1a", bufs=2)
                            for i in range(2)]

                ones_b = sb.tile([128, 128], bf, tag="ones", bufs=2)
                nc.vector.memset(ones_b[:], 1.0)
                eps_ap = sb.tile([128, 1], f32, tag="eps", bufs=1)
                nc.vector.memset(eps_ap[:], LN_EPS)

                # ---- P1: LN1, pipelined by token half: tb0's stats, stats
                # chain and fp8 activations complete (unblocking Q/V and the
                # first pairs) while tb1 is still streaming in ----
                s1t = psum2()
                s2t = psum2()
                ps_s1 = [s1t[:, 0:512], s1t[:, 512:1024]]
                ps_s2 = [s2t[:, 0:512], s2t[:, 512:1024]]
                mu01 = sb.tile([128, 1024], bf, tag="stats16", bufs=3, name="mu01")
                rstd01 = sb.tile([128, 1024], bf, tag="stats16", bufs=3, name="rstd01")
                g_all = sb.tile([128, KO, S], f8, tag="gall", bufs=1, name="gall")
                for tb in range(2):
                    for ko in range(KO):
                        sqk = sb.tile([128, Q], bf, tag="qslab", bufs=5, name="sqk")
                        nc.vector.tensor_tensor(sqk[:], xb8[ko][:, ts(tb, 512)],
                                                xb8[ko][:, ts(tb, 512)], ALU.mult)
                        nc.tensor.matmul(ps_s1[tb], ones_b[:],
                                         xb8[ko][:, ts(tb, 512)],
                                         start=(ko == 0), stop=(ko == KO - 1))
                        nc.tensor.matmul(ps_s2[tb], ones_b[:], sqk[:],
                                         start=(ko == 0), stop=(ko == KO - 1))
                    with nc.allow_low_precision(reason="bf16 LN mean"):
                        nc.vector.tensor_scalar_mul(mu01[:, ts(tb, 512)],
                                                    ps_s1[tb], 1.0 / D)
                    ex2 = tmpf()
                    nc.vector.tensor_scalar_mul(ex2[:], ps_s2[tb], 1.0 / D)
                    var = tmpf()
                    nc.vector.tensor_tensor(var[:], mu01[:, ts(tb, 512)],
                                            mu01[:, ts(tb, 512)], ALU.mult)
                    nc.vector.tensor_tensor(var[:], ex2[:], var[:], ALU.subtract)
                    sd = tmpf()
                    nc.scalar.activation(sd[:], var[:], AF.Sqrt, bias=eps_ap[:])
                    with nc.allow_low_precision(reason="bf16 LN rstd"):
                        nc.vector.reciprocal(rstd01[:, ts(tb, 512)], sd[:])
                    # normalized activations for this half: bf16 on DVE (2x),
                    # fp8 cast on Act
                    for ko in range(KO):
                        tm = sb.tile([128, Q], bf, tag="qslab", bufs=5, name="tm")
                        nc.vector.tensor_tensor(tm[:], xb8[ko][:, ts(tb, 512)],
                                                mu01[:, ts(tb, 512)], ALU.subtract)
                        gb = sb.tile([128, Q], bf, tag="qslab", bufs=5, name="gb")
                        nc.vector.tensor_tensor(gb[:], tm[:],
                                                rstd01[:, ts(tb, 512)], ALU.mult)
                        with nc.allow_low_precision(reason="fp8 activations"):
                            nc.scalar.copy(g_all[:, ko, ts(tb, 512)], gb[:])

                # ---- P2: V projection (fp8 DR), token-major, + ones columns
                # at 64 (even head of pair) and 65 (odd head) for the softmax
                # denominator trick ----
                v_sb = sb.tile([128, KO, H, 66], bf, tag="m16v", bufs=1, name="vsb")
                nc.vector.memset(v_sb[:, :, :, 64:66], 1.0)

                def vproj(nb, tos):
                    for to in tos:
                        pv = psum()
                        for kp in range(KO // 2):
                            nc.tensor.matmul(pv[:],
                                             g_all[:, 2 * kp:2 * kp + 2, ts(to, 128)],
                                             wv_sb[nb][:, 2 * kp:2 * kp + 2, :],
                                             start=(kp == 0), stop=(kp == KO // 2 - 1),
                                             perf_mode=DR)
                        nc.scalar.activation(
                            v_sb[:, to, nb * 8:(nb + 1) * 8, 0:64],
                            pv[:].rearrange("p (h d) -> p h d", d=64),
                            AF.Identity, scale=1.0 / SW)

                # ---- P3: per-head-pair software pipeline ----
                wo_sb = [wpiece(wo_d, i, [128, KO, 512], dt=f8) for i in range(2)]
                oT_all = sb.tile([128, KO, Q], f8, tag="oTall", bufs=1, name="oTall")

                def swap_dma(dst, src, n):
                    # swap 32-row halves within each 64-row (head) block
                    for gI in range(2):
                        r = gI * 64
                        nc.sync.dma_start(dst[r:r + 32, :n], src[r + 32:r + 64, :n])
                        nc.sync.dma_start(dst[r + 32:r + 64, :n], src[r:r + 32, :n])

                def proj_rope(jo):
                    """PE: 12 fp8-DR matmuls; DVE: bias + rope; -> (qr, kr)."""
                    pq = psum()
                    for kp in range(KO // 2):
                        nc.tensor.matmul(pq[:],
                                         wq_sb[jo // 4][:, 2 * kp:2 * kp + 2, ts(jo % 4, 128)],
                                         g_all[:, 2 * kp:2 * kp + 2, 0:Q],
                                         start=(kp == 0), stop=(kp == KO // 2 - 1),
                                         perf_mode=DR)
                    pks = []
                    for tb in range(2):
                        pk = psum()
                        for kp in range(KO // 2):
                            nc.tensor.matmul(pk[:],
                                             wk_sb[jo // 4][:, 2 * kp:2 * kp + 2, ts(jo % 4, 128)],
                                             g_all[:, 2 * kp:2 * kp + 2, ts(tb, 512)],
                                             start=(kp == 0), stop=(kp == KO // 2 - 1),
                                             perf_mode=DR)
                        pks.append(pk)
                    qa = sb.tile([128, 512], bf, tag="qslab", bufs=5, name="qa")
                    ka = sb.tile([128, S], bf, tag="kslab", bufs=4, name="ka")
                    if USE_TS:
                        nc.vector.tensor_scalar(qa[:], pq[:], 1.0 / SW,
                                                bq_s[:, jo:jo + 1], ALU.mult, ALU.add)
                        for tb in range(2):
                            nc.vector.tensor_scalar(ka[:, ts(tb, 512)], pks[tb][:],
                                                    1.0 / SW, bk_s[:, jo:jo + 1],
                                                    ALU.mult, ALU.add)
                    else:
                        nc.scalar.activation(qa[:], pq[:], AF.Identity,
                                             bias=bq_s[:, jo:jo + 1], scale=1.0 / SW)
                        for tb in range(2):
                            nc.scalar.activation(ka[:, ts(tb, 512)], pks[tb][:],
                                                 AF.Identity,
                                                 bias=bk_s[:, jo:jo + 1],
                                                 scale=1.0 / SW)
                    qsw = sb.tile([128, 512], bf, tag="qslab", bufs=5, name="qsw")
                    swap_dma(qsw, qa, Q)
                    t1 = sb.tile([128, 512], bf, tag="qslab", bufs=5, name="qt1")
                    nc.vector.tensor_tensor(t1[:], qa[:], csb[:, 0:Q], ALU.mult)
                    t2 = sb.tile([128, 512], bf, tag="qslab", bufs=5, name="qt2")
                    nc.vector.tensor_tensor(t2[:], qsw[:], ssb[:, 0:Q], ALU.mult)
                    qr = sb.tile([128, Q], bf, tag="act1k", bufs=10, name="qr")
                    nc.vector.tensor_tensor(qr[:], t1[:], t2[:], ALU.add)

                    ksw = sb.tile([128, S], bf, tag="kslab", bufs=4, name="ksw")
                    swap_dma(ksw, ka, S)
                    kt1 = sb.tile([128, S], bf, tag="kslab", bufs=4, name="kt1")
                    nc.vector.tensor_tensor(kt1[:], ka[:], csb[:], ALU.mult)
                    nc.vector.tensor_tensor(ksw[:], ksw[:], ssb[:], ALU.mult)
                    kr = sb.tile([128, S], bf, tag="act2k", bufs=10, name="kr")
                    nc.vector.tensor_tensor(kr[:], kt1[:], ksw[:], ALU.add)
                    return qr, kr

                # prologue: pair 0's projections first, then V fills the PE
                # while DVE ropes pair 0
                qk = {0: proj_rope(0)}
                vproj(0, range(KO))
                for hp in range(8):
                    jo = hp
                    qr, kr = qk.pop(hp)
                    # scores + exp (probs bf16), 4 chunks of 2 key-tiles.
                    # attnV halves trail the scores by 2 so PE chews attnV
                    # while Act exponentiates the freshly produced scores.
                    probs = {0: [], 1: []}
                    po2 = {0: psumv(), 1: psumv()}

                    def scores_half(half):
                        pbig = {}
                        for sub in range(2):
                            r0 = sub * 64
                            big = psum2()
                            for kk in range(2):
                                kt = half * 2 + kk
                                nc.tensor.matmul(big[:, ts(kk, 512)],
                                                 kr[r0:r0 + 64, ts(kt, 128)],
                                                 qr[r0:r0 + 64, :],
                                                 start=True, stop=True,
                                                 tile_position=(r0, 0))
                            pbig[sub] = big
                        for sub in range(2):
                            pb = sb.tile([128, 1024], bf, tag="probs", bufs=5, name="pb")
                            nc.scalar.activation(pb[:], pbig[sub][:], AF.Exp,
                                                 scale=0.125)
                            probs[sub].append(pb)

                    def attnv_half(half):
                        for sub in range(2):
                            h = 2 * hp + sub
                            for kk in range(2):
                                kt = 2 * half + kk
                                nc.tensor.matmul(po2[sub][0:65, :],
                                                 v_sb[:, kt, h, 0:65],
                                                 probs[sub][half][:, ts(kk, 512)],
                                                 start=(kt == 0), stop=(kt == KO - 1))

                    for half in range(4):
                        scores_half(half)
                        if half >= 2:
                            attnv_half(half - 2)
                    # filler while exp(h2/h3) complete: next pair's
                    # projections + rope, and the spread 2nd V half
                    if hp + 1 < 8:
                        qk[hp + 1] = proj_rope(hp + 1)
                    if 1 <= hp <= 4:
                        vproj(1, (2 * (hp - 1), 2 * (hp - 1) + 1))
                    attnv_half(2)
                    attnv_half(3)
                    if USE_PAIR_DEN:
                        # pair denominator: rcp rows 64/65, one broadcast matmul
                        den2 = sb.tile([66, 512], bf, tag="rcp", bufs=2, name="den2")
                        with nc.allow_low_precision(reason="bf16 softmax denom"):
                            nc.vector.reciprocal(den2[64:65, :], po2[0][64:65, :])
                            nc.vector.reciprocal(den2[65:66, :], po2[1][65:66, :])
                        prb = psum()
                        nc.tensor.matmul(prb[:], selp[64:66, :], den2[64:66, :],
                                         start=True, stop=True)
                        for sub in range(2):
                            r0 = sub * 64
                            o16 = sb.tile([64, 512], f8, tag="o16", bufs=2, name="o16")
                            with nc.allow_low_precision(reason="fp8 attn output"):
                                nc.vector.tensor_tensor(o16[:], po2[sub][0:64, :],
                                                        prb[r0:r0 + 64, :], ALU.mult)
                            nc.sync.dma_start(oT_all[r0:r0 + 64, jo, :], o16[:])
                    else:
                        for sub in range(2):
                            r0 = sub * 64
                            po = po2[sub]
                            rcp = sb.tile([65, 512], bf, tag="rcp", bufs=2, name="rcp")
                            with nc.allow_low_precision(reason="bf16 softmax denom"):
                                nc.vector.reciprocal(rcp[64:65, :], po[64:65, :])
                            prb = psum()
                            nc.tensor.matmul(prb[0:64, :], ones_b[64:65, 0:64],
                                             rcp[64:65, :], start=True, stop=True)
                            rb = sb.tile([64, 512], bf, tag="rb", bufs=2, name="rb")
                            nc.vector.tensor_copy(rb[:], prb[0:64, :])
                            o16 = sb.tile([64, 512], f8, tag="o16", bufs=2, name="o16")
                            with nc.allow_low_precision(reason="fp8 attn output"):
                                nc.vector.tensor_tensor(o16[:], po[0:64, :],
                                                        rb[:], ALU.mult)
                            nc.sync.dma_start(oT_all[r0:r0 + 64, jo, :], o16[:])

                # ---- P4: attn out (fp8 DR) + gated residual (bf16), fused
                # with LN2 stats so DVE/PE overlap per do-chunk ----
                x2 = []
                p12 = psum2()
                p1 = p12[:, 0:512]
                p2 = p12[:, 512:1024]
                for do in range(KO):
                    py = psum()
                    for kp in range(KO // 2):
                        nc.tensor.matmul(py[:],
                                         wo_sb[do // 4][:, 2 * kp:2 * kp + 2, ts(do % 4, 128)],
                                         oT_all[:, 2 * kp:2 * kp + 2, :],
                                         start=(kp == 0), stop=(kp == KO // 2 - 1),
                                         perf_mode=DR)
                    t = sb.tile([128, Q], bf, tag="qslab", bufs=5, name="tao")
                    nc.scalar.activation(t[:], py[:], AF.Identity,
                                         bias=bo_s[:, do:do + 1],
                                         scale=gm_s[:, do:do + 1])
                    xx = sb.tile([128, Q], bf, tag="act2k", bufs=10, name="x2")
                    nc.vector.tensor_tensor(xx[:], t[:], xb8[do][:, 0:Q], ALU.add)
                    x2.append(xx)
                    sq2 = sb.tile([128, Q], bf, tag="qslab", bufs=5, name="sq2")
                    nc.vector.tensor_tensor(sq2[:], xx[:], xx[:], ALU.mult)
                    nc.tensor.matmul(p1, ones_b[:], xx[:], start=(do == 0),
                                     stop=(do == KO - 1))
                    nc.tensor.matmul(p2, ones_b[:], sq2[:], start=(do == 0),
                                     stop=(do == KO - 1))

                # ---- P5: LN2 tail ----
                mu16 = sb.tile([128, 512], bf, tag="stats16", bufs=3, name="mu16b")
                with nc.allow_low_precision(reason="bf16 LN mean"):
                    nc.vector.tensor_scalar_mul(mu16[:], p1, 1.0 / D)
                ex2 = tmpf()
                nc.vector.tensor_scalar_mul(ex2[:], p2, 1.0 / D)
                var = tmpf()
                nc.vector.tensor_tensor(var[:], mu16[:], mu16[:], ALU.mult)
                nc.vector.tensor_tensor(var[:], ex2[:], var[:], ALU.subtract)
                sd2 = tmpf()
                nc.scalar.activation(sd2[:], var[:], AF.Sqrt, bias=eps_ap[:])
                rstd16 = sb.tile([128, 512], bf, tag="stats16", bufs=3, name="rstd16b")
                with nc.allow_low_precision(reason="bf16 LN rstd"):
                    nc.vector.reciprocal(rstd16[:], sd2[:])
                g2 = []
                for ko in range(KO):
                    tm2 = sb.tile([128, 512], bf, tag="qslab", bufs=5, name="tm2")
                    nc.vector.tensor_tensor(tm2[:], x2[ko][:], mu16[:], ALU.subtract)
                    gk = sb.tile([128, Q], bf, tag="act1k", bufs=10, name="g2")
                    nc.vector.tensor_tensor(gk[:], tm2[:], rstd16[:], ALU.mult)
                    g2.append(gk)

                # ---- P6/P7: MLP (bf16) ----
                w1_sb = w1_early + [wpiece(w1_d, i, [128, KO, 512])
                                    for i in range(2, 8)]
                w2_sb = [wpiece(w2_d, i, [128, 32, 128]) for i in range(8)]
                m16 = sb.tile([128, 32, Q], bf, tag="m16v", bufs=1, name="m16")
                for mo in range(32):
                    pm = psum()
                    for ko in range(KO):
                        nc.tensor.matmul(pm[:], w1_sb[mo // 4][:, ko, ts(mo % 4, 128)],
                                         g2[ko][:], start=(ko == 0),
                                         stop=(ko == KO - 1))
                    nc.scalar.activation(m16[:, mo], pm[:], AF.Gelu_apprx_tanh,
                                         bias=b1_s[:, mo:mo + 1], scale=1.0)
                yt_r = yt_d.rearrange("(ko p) t -> p ko t", p=128)
                for do in range(KO):
                    pz = psum()
                    for ko in range(32):
                        nc.tensor.matmul(pz[:], w2_sb[do][:, ko, :],
                                         m16[:, ko, :], start=(ko == 0), stop=(ko == 31))
                    t = tmpf()
                    nc.scalar.activation(t[:], pz[:], AF.Identity,
                                         bias=b2_s[:, do:do + 1],
                                         scale=gp_s[:, do:do + 1])
                    yk = sb.tile([128, Q], f32, tag="yout", bufs=4, name="yout")
                    nc.vector.tensor_tensor(yk[:], t[:], x2[do][:], ALU.add)
                    for tb in range(2):
                        nc.sync.dma_start(yt_r[:, do, ts(tb, 256)],
                                          yk[:, ts(tb, 256)])

    nc.compile()
    return nc


# ----------------------------------------------------------------------------
# host wrapper
# ----------------------------------------------------------------------------

def _prep_shared(inputs):
    x = np.asarray(inputs["x"], np.float32)
    c = np.asarray(inputs["c"], np.float32)
    w_ada = np.asarray(inputs["w_ada"], np.float32)
    b_ada = np.asarray(inputs["b_ada"], np.float32)
    w_qkv = np.asarray(inputs["w_qkv"], np.float32)
    w_ao = np.asarray(inputs["w_attn_out"], np.float32)
    w_m1 = np.asarray(inputs["w_mlp1"], np.float32)
    w_m2 = np.asarray(inputs["w_mlp2"], np.float32)

    mod = c @ w_ada + b_ada
    sh_msa, sc_msa, g_msa, sh_mlp, sc_mlp, g_mlp = np.split(mod, 6, axis=1)
    ln1 = np.asarray(inputs["w_ln1"], np.float32) * (1.0 + sc_msa)   # [4, D]
    ln2 = np.asarray(inputs["w_ln2"], np.float32) * (1.0 + sc_mlp)

    shared = {}
    for b in range(B):
        Wq = w_qkv[:, :D] * ln1[b][:, None]
        Wk = w_qkv[:, D:2 * D] * ln1[b][:, None]
        Wv = w_qkv[:, 2 * D:] * ln1[b][:, None]
        bqkv = sh_msa[b] @ w_qkv
        W1 = w_m1 * ln2[b][:, None]
        bm1 = sh_mlp[b] @ w_m1 + np.asarray(inputs["b_mlp1"], np.float32)
        bvec = np.concatenate([
            _pvec(bqkv[:D]), _pvec(bqkv[D:2 * D]),
            _pvec((bqkv[2 * D:] @ w_ao) * g_msa[b]),
            _pvec(g_msa[b] / SW), _pvec(bm1),
            _pvec(np.asarray(inputs["b_mlp2"], np.float32) * g_mlp[b]),
            _pvec(g_mlp[b]),
        ], axis=1)
        shared[b] = dict(
            wq=_pieces(Wq, 512, F8, SW), wk=_pieces(Wk, 512, F8, SW),
            wv=_pieces(Wv, 512, F8, SW),
            wm1=_pieces(W1, 512),
            bvec=np.ascontiguousarray(bvec),
        )
    wao_p = _pieces(w_ao, 512, F8, SW)
    wm2_p = _pieces(w_m2, 128)
    bm2_p = _pvec(np.asarray(inputs["b_mlp2"], np.float32))
    cos = np.asarray(inputs["cos"], np.float32)
    sin = np.asarray(inputs["sin"], np.float32)
    return shared, wao_p, wm2_p, bm2_p, x, cos, sin


def _make_in_maps(inputs):
    shared, wao_p, wm2_p, bm2_p, x, cos, sin = _prep_shared(inputs)
    in_maps = []
    for core in range(8):
        b, half = core // 2, core % 2
        qlo = half * Q
        order = np.concatenate([np.arange(qlo, qlo + Q), np.arange(0, qlo),
                                np.arange(qlo + Q, S)])
        xT = x[b][order].T
        cosT = cos[order].T                      # [32, S]
        sinT = sin[order].T
        cc = np.concatenate([cosT] * 4, 0).astype(BF)
        ss = np.concatenate([-sinT, sinT, -sinT, sinT], 0).astype(BF)
        sh = shared[b]
        in_maps.append({
            "xb": np.ascontiguousarray(xT.astype(BF)),
            "wq": sh["wq"], "wk": sh["wk"], "wv": sh["wv"],
            "wao": wao_p, "wm1": sh["wm1"], "wm2": wm2_p,
            "cc": np.ascontiguousarray(cc), "ss": np.ascontiguousarray(ss),
            "bvec": sh["bvec"],
        })
    return in_maps


def kernel(**inputs):
    from concourse import bass_utils

    if "nc" not in _CACHE:
        _CACHE["nc"] = _build_program()
    nc = _CACHE["nc"]

    in_maps = _make_in_maps(inputs)
    res = bass_utils.run_bass_kernel_spmd(nc, in_maps, core_ids=list(range(8)))

    y = np.zeros((B, S, D), np.float32)
    for core in range(8):
        b, half = core // 2, core % 2
        qlo = half * Q
        y[b, qlo:qlo + Q] = res.results[core]["yt"].T
    return y


# revision 3
# speedup vs baseline: 1.1528x; 1.0040x over previous
"""DDiT block kernel for 8 Trainium2 NeuronCores.

Sharding: data-parallel over (batch, sequence-half) -> 8 shards. Each core
processes one batch's full sequence through LN1/K/V (needed for attention),
but only its 512 query tokens through Q/attention/MLP.

Device layout is feature-major (model dim on partitions, tokens on the free
axis), which makes every adaLN modulation a per-partition scalar and lets all
matmuls consume activations without transposes. Host folds the adaLN scale
and LN weight into the qkv/mlp1 weights, and the shift-vectors into biases.
Tokens are rotated per-core so queries are always tokens [0:512).

QKV / attn-out projections run in fp8e4m3 DoubleRow (weights pre-scaled by
SW, compensated on PSUM read-out); scores, probs, attnV and the MLP stay
bf16 (fp8 there breaks the 2e-2 gate). LN stats, softmax and residuals are
fp32. The attention region is a per-head-pair software pipeline so PE
(matmuls), Act (exp) and DVE (rope, bias, normalize) overlap.
"""

import numpy as np
import ml_dtypes

BF = ml_dtypes.bfloat16
F8 = ml_dtypes.float8_e4m3

B, S, D, H, HD = 4, 1024, 1024, 16, 64
Q = 512          # queries per core
KO = 8           # 1024 dim / 128 partitions
MLP = 4096
LN_EPS = 1e-5
SW = 64.0        # fp8 weight pre-scale (compensated on PSUM read-out)

_CACHE = {}

# bisect flags (compile-crash isolation)
USE_TS = True         # DVE tensor_scalar for q/k bias (else Act activation)
USE_PAIR_DEN = False  # dead: partition ranges must start at 0/32/64/96


# ----------------------------------------------------------------------------
# host-side layout helpers
# ----------------------------------------------------------------------------

def _pieces(W, m_piece, dt=BF, scale=1.0):
    """[K, M] weight -> [n_pieces, 128, K//128, m_piece], contiguous."""
    K, M = W.shape
    ko = K // 128
    Wr = (np.asarray(W, np.float32) * scale).reshape(ko, 128, M).transpose(1, 0, 2)
    n = M // m_piece
    out = Wr.reshape(128, ko, n, m_piece).transpose(2, 0, 1, 3)
    return np.ascontiguousarray(out.astype(dt))


def _pvec(v):
    """[M] vector -> [128, M//128] f32 (partition-major chunks)."""
    v = np.asarray(v, np.float32)
    return np.ascontiguousarray(v.reshape(-1, 128).T)


# ----------------------------------------------------------------------------
# device program
# ----------------------------------------------------------------------------

def _build_program(repeat=1):
    import concourse.bass as bass
    import concourse.mybir as mybir
    import concourse.tile as tile
    from concourse import bacc

    f32 = mybir.dt.float32
    bf = mybir.dt.bfloat16
    f8 = mybir.dt.float8e4
    AF = mybir.ActivationFunctionType
    ALU = mybir.AluOpType
    DR = mybir.MatmulPerfMode.DoubleRow
    ts = bass.ts

    nc = bacc.Bacc("TRN2", target_bir_lowering=False, debug=False,
                   enable_asserts=False)

    def din(name, shape, dt=bf):
        return nc.dram_tensor(name, shape, dt, kind="ExternalInput").ap()

    xb_d = din("xb", [D, S])                      # bf16 x, feature-major
    wq_d = din("wq", [2, 128, KO, 512], f8)
    wk_d = din("wk", [2, 128, KO, 512], f8)
    wv_d = din("wv", [2, 128, KO, 512], f8)
    wo_d = din("wao", [2, 128, KO, 512], f8)
    w1_d = din("wm1", [8, 128, KO, 512])
    w2_d = din("wm2", [8, 128, 32, 128])
    cc_d = din("cc", [128, S])
    ss_d = din("ss", [128, S])
    bv_d = din("bvec", [128, 80], f32)
    yt_d = nc.dram_tensor("yt", [D, Q], f32, kind="ExternalOutput").ap()

    with tile.TileContext(nc) as tc:
        with tc.tile_pool(name="sb", bufs=1) as sb, \
             tc.tile_pool(name="ps", bufs=1, space="PSUM") as ps:
            for _rep in range(repeat):

                def psum():
                    return ps.tile([128, 512], f32, tag="p", bufs=2, name="pt")

                def psumv():
                    return ps.tile([128, 512], f32, tag="pv", bufs=2, name="pvt")

                def psum2():
                    return ps.tile([128, 1024], f32, tag="p2", bufs=2, name="pt2")

                def tmpf():
                    return sb.tile([128, 512], f32, tag="tmpf", bufs=3, name="tf")

                # ---- P0: DMA schedule. xb token-half tb0 first (alternating
                # SP/Act issue queues) so LN1-tb0 unblocks projections while
                # tb1 and the relaxed weights stream in behind. ----
                xb_r = xb_d.rearrange("(ko p) t -> p ko t", p=128)
                xb8 = [sb.tile([128, S], bf, tag="xb8", bufs=8, name="xb")
                       for _ in range(KO)]
                for ko in range(KO):
                    nc.sync.dma_start(xb8[ko][:, 0:512], xb_r[:, ko, 0:512])

                def wpiece(dram, i, shape, tag="w8", bufs=6, dt=bf, eng=None,
                           nsplit=2):
                    # split along the ko axis so DMA engines share a piece
                    eng = eng or nc.sync
                    t = sb.tile(shape, dt, tag=tag, bufs=bufs, name="w")
                    h = shape[1] // nsplit
                    for j in range(nsplit):
                        eng.dma_start(t[:, j * h:(j + 1) * h],
                                      dram[i][:, j * h:(j + 1) * h])
                    return t

                # earliest-needed weights as quarter transfers
                wq_sb = [None, None]
                wk_sb = [None, None]
                wv_sb = [None, None]
                wq_sb[0] = wpiece(wq_d, 0, [128, KO, 512], dt=f8, nsplit=4)
                csb = sb.tile([128, S], bf, tag="cs", bufs=2)
                ssb = sb.tile([128, S], bf, tag="cs", bufs=2)
                for j in range(2):
                    nc.sync.dma_start(csb[:, ts(j, 512)], cc_d[:, ts(j, 512)])
                    nc.sync.dma_start(ssb[:, ts(j, 512)], ss_d[:, ts(j, 512)])
                for ko in range(KO):
                    nc.sync.dma_start(xb8[ko][:, 512:1024], xb_r[:, ko, 512:1024])
                wk_sb[0] = wpiece(wk_d, 0, [128, KO, 512], dt=f8, nsplit=4)
                wv_sb[0] = wpiece(wv_d, 0, [128, KO, 512], dt=f8, nsplit=4)
                bvec = sb.tile([128, 80], f32, tag="bias", bufs=1, name="bvec")
                nc.sync.dma_start(bvec[:], bv_d[:])
                bq_s, bk_s, bo_s, gm_s = (bvec[:, 8 * i:8 * i + 8]
                                          for i in range(4))
                b1_s = bvec[:, 32:64]
                b2_s, gp_s = bvec[:, 64:72], bvec[:, 72:80]
                wq_sb[1] = wpiece(wq_d, 1, [128, KO, 512], dt=f8)
                wk_sb[1] = wpiece(wk_d, 1, [128, KO, 512], dt=f8)
                wv_sb[1] = wpiece(wv_d, 1, [128, KO, 512], dt=f8)
                # first two MLP1 weight pieces prefetch into dedicated buffers
                # (the w8 ring + SP queue only frees up after the pairs)
                w1_early = [wpiece(w1_d, i, [128, KO, 512], tag="w1a", bufs=2)
                            for i in range(2)]

                ones_b = sb.tile([128, 128], bf, tag="ones", bufs=2)
                nc.vector.memset(ones_b[:], 1.0)
                eps_ap = sb.tile([128, 1], f32, tag="eps", bufs=1)
                nc.vector.memset(eps_ap[:], LN_EPS)

                # ---- P1: LN1, pipelined by token half: tb0's stats, stats
                # chain and fp8 activations complete (unblocking Q/V and the
                # first pairs) while tb1 is still streaming in ----
                s1t = psum2()
                s2t = psum2()
                ps_s1 = [s1t[:, 0:512], s1t[:, 512:1024]]
                ps_s2 = [s2t[:, 0:512], s2t[:, 512:1024]]
                mu01 = sb.tile([128, 1024], bf, tag="stats16", bufs=3, name="mu01")
                rstd01 = sb.tile([128, 1024], bf, tag="stats16", bufs=3, name="rstd01")
                g_all = sb.tile([128, KO, S], f8, tag="gall", bufs=1, name="gall")
                for tb in range(2):
                    for ko in range(KO):
                        sqk = sb.tile([128, Q], bf, tag="qslab", bufs=5, name="sqk")
                        nc.vector.tensor_tensor(sqk[:], xb8[ko][:, ts(tb, 512)],
                                                xb8[ko][:, ts(tb, 512)], ALU.mult)
                        nc.tensor.matmul(ps_s1[tb], ones_b[:],
                                         xb8[ko][:, ts(tb, 512)],
                                         start=(ko == 0), stop=(ko == KO - 1))
                        nc.tensor.matmul(ps_s2[tb], ones_b[:], sqk[:],
                                         start=(ko == 0), stop=(ko == KO - 1))
                    with nc.allow_low_precision(reason="bf16 LN mean"):
                        nc.vector.tensor_scalar_mul(mu01[:, ts(tb, 512)],
                                                    ps_s1[tb], 1.0 / D)
                    ex2 = tmpf()
                    nc.vector.tensor_scalar_mul(ex2[:], ps_s2[tb], 1.0 / D)
                    var = tmpf()
                    nc.vector.tensor_tensor(var[:], mu01[:, ts(tb, 512)],
                                            mu01[:, ts(tb, 512)], ALU.mult)
                    nc.vector.tensor_tensor(var[:], ex2[:], var[:], ALU.subtract)
                    sd = tmpf()
                    nc.scalar.activation(sd[:], var[:], AF.Sqrt, bias=eps_ap[:])
                    with nc.allow_low_precision(reason="bf16 LN rstd"):
                        nc.vector.reciprocal(rstd01[:, ts(tb, 512)], sd[:])
                    # normalized activations for this half: bf16 on DVE (2x),
                    # fp8 cast on Act
                    for ko in range(KO):
                        tm = sb.tile([128, Q], bf, tag="qslab", bufs=5, name="tm")
                        nc.vector.tensor_tensor(tm[:], xb8[ko][:, ts(tb, 512)],
                                                mu01[:, ts(tb, 512)], ALU.subtract)
                        gb = sb.tile([128, Q], bf, tag="qslab", bufs=5, name="gb")
                        nc.vector.tensor_tensor(gb[:], tm[:],
                                                rstd01[:, ts(tb, 512)], ALU.mult)
                        with nc.allow_low_precision(reason="fp8 activations"):
                            nc.scalar.copy(g_all[:, ko, ts(tb, 512)], gb[:])

                # ---- P2: V projection (fp8 DR), token-major, + ones columns
                # at 64 (even head of pair) and 65 (odd head) for the softmax
                # denominator trick ----
                v_sb = sb.tile([128, KO, H, 66], bf, tag="m16v", bufs=1, name="vsb")
                nc.vector.memset(v_sb[:, :, :, 64:66], 1.0)

                def vproj(nb, tos):
                    for to in tos:
                        pv = psum()
                        for kp in range(KO // 2):
                            nc.tensor.matmul(pv[:],
                                             g_all[:, 2 * kp:2 * kp + 2, ts(to, 128)],
                                             wv_sb[nb][:, 2 * kp:2 * kp + 2, :],
                                             start=(kp == 0), stop=(kp == KO // 2 - 1),
                                             perf_mode=DR)
                        nc.scalar.activation(
                            v_sb[:, to, nb * 8:(nb + 1) * 8, 0:64],
                            pv[:].rearrange("p (h d) -> p h d", d=64),
                            AF.Identity, scale=1.0 / SW)

                # ---- P3: per-head-pair software pipeline ----
                wo_sb = [wpiece(wo_d, i, [128, KO, 512], dt=f8) for i in range(2)]
                oT_all = sb.tile([128, KO, Q], f8, tag="oTall", bufs=1, name="oTall")

                def swap_dma(dst, src, n):
                    # swap 32-row halves within each 64-row (head) block
                    for gI in range(2):
                        r = gI * 64
                        nc.sync.dma_start(dst[r:r + 32, :n], src[r + 32:r + 64, :n])
                        nc.sync.dma_start(dst[r + 32:r + 64, :n], src[r:r + 32, :n])

                def proj_rope(jo):
                    """PE: 12 fp8-DR matmuls; DVE: bias + rope; -> (qr, kr)."""
                    pq = psum()
                    for kp in range(KO // 2):
                        nc.tensor.matmul(pq[:],
                                         wq_sb[jo // 4][:, 2 * kp:2 * kp + 2, ts(jo % 4, 128)],
                                         g_all[:, 2 * kp:2 * kp + 2, 0:Q],
                                         start=(kp == 0), stop=(kp == KO // 2 - 1),
                                         perf_mode=DR)
                    pks = []
                    for tb in range(2):
                        pk = psum()
                        for kp in range(KO // 2):
                            nc.tensor.matmul(pk[:],
                                             wk_sb[jo // 4][:, 2 * kp:2 * kp + 2, ts(jo % 4, 128)],
                                             g_all[:, 2 * kp:2 * kp + 2, ts(tb, 512)],
                                             start=(kp == 0), stop=(kp == KO // 2 - 1),
                                             perf_mode=DR)
                        pks.append(pk)
                    qa = sb.tile([128, 512], bf, tag="qslab", bufs=5, name="qa")
                    ka = sb.tile([128, S], bf, tag="kslab", bufs=4, name="ka")
                    if USE_TS:
                        nc.vector.tensor_scalar(qa[:], pq[:], 1.0 / SW,
                                                bq_s[:, jo:jo + 1], ALU.mult, ALU.add)
                        for tb in range(2):
                            nc.vector.tensor_scalar(ka[:, ts(tb, 512)], pks[tb][:],
                                                    1.0 / SW, bk_s[:, jo:jo + 1],
                                                    ALU.mult, ALU.add)
                    else:
                        nc.scalar.activation(qa[:], pq[:], AF.Identity,
                                             bias=bq_s[:, jo:jo + 1], scale=1.0 / SW)
                        for tb in range(2):
                            nc.scalar.activation(ka[:, ts(tb, 512)], pks[tb][:],
                                                 AF.Identity,
                                                 bias=bk_s[:, jo:jo + 1],
                                                 scale=1.0 / SW)
                    qsw = sb.tile([128, 512], bf, tag="qslab", bufs=5, name="qsw")
                    swap_dma(qsw, qa, Q)
                    t1 = sb.tile([128, 512], bf, tag="qslab", bufs=5, name="qt1")
                    nc.vector.tensor_tensor(t1[:], qa[:], csb[:, 0:Q], ALU.mult)
                    t2 = sb.tile([128, 512], bf, tag="qslab", bufs=5, name="qt2")
                    nc.vector.tensor_tensor(t2[:], qsw[:], ssb[:, 0:Q], ALU.mult)
                    qr = sb.tile([128, Q], bf, tag="act1k", bufs=10, name="qr")
                    nc.vector.tensor_tensor(qr[:], t1[:], t2[:], ALU.add)

                    ksw = sb.tile([128, S], bf, tag="kslab", bufs=4, name="ksw")
                    swap_dma(ksw, ka, S)
                    kt1 = sb.tile([128, S], bf, tag="kslab", bufs=4, name="kt1")
                    nc.vector.tensor_tensor(kt1[:], ka[:], csb[:], ALU.mult)
                    nc.vector.tensor_tensor(ksw[:], ksw[:], ssb[:], ALU.mult)
                    kr = sb.tile([128, S], bf, tag="act2k", bufs=10, name="kr")
                    nc.vector.tensor_tensor(kr[:], kt1[:], ksw[:], ALU.add)
                    return qr, kr

                # prologue: pair 0's projections first, then V fills the PE
                # while DVE ropes pair 0
                qk = {0: proj_rope(0)}
                vproj(0, range(KO))
                for hp in range(8):
                    jo = hp
                    qr, kr = qk.pop(hp)
                    # scores + exp (probs bf16), 4 chunks of 2 key-tiles.
                    # attnV halves trail the scores by 2 so PE chews attnV
                    # while Act exponentiates the freshly produced scores.
                    probs = {0: [], 1: []}
                    po2 = {0: psumv(), 1: psumv()}

                    def scores_half(half):
                        pbig = {}
                        for sub in range(2):
                            r0 = sub * 64
                            big = psum2()
                            for kk in range(2):
                                kt = half * 2 + kk
                                nc.tensor.matmul(big[:, ts(kk, 512)],
                                                 kr[r0:r0 + 64, ts(kt, 128)],
                                                 qr[r0:r0 + 64, :],
                                                 start=True, stop=True,
                                                 tile_position=(r0, 0))
                            pbig[sub] = big
                        for sub in range(2):
                            pb = sb.tile([128, 1024], bf, tag="probs", bufs=5, name="pb")
                            nc.scalar.activation(pb[:], pbig[sub][:], AF.Exp,
                                                 scale=0.125)
                            probs[sub].append(pb)

                    def attnv_half(half):
                        for sub in range(2):
                            h = 2 * hp + sub
                            for kk in range(2):
                                kt = 2 * half + kk
                                nc.tensor.matmul(po2[sub][0:65, :],
                                                 v_sb[:, kt, h, 0:65],
                                                 probs[sub][half][:, ts(kk, 512)],
                                                 start=(kt == 0), stop=(kt == KO - 1))

                    for half in range(4):
                        scores_half(half)
                        if half >= 2:
                            attnv_half(half - 2)
                    # filler while exp(h2/h3) complete: next pair's
                    # projections + rope, and the spread 2nd V half
                    if hp + 1 < 8:
                        qk[hp + 1] = proj_rope(hp + 1)
                    if 1 <= hp <= 4:
                        vproj(1, (2 * (hp - 1), 2 * (hp - 1) + 1))
                    attnv_half(2)
                    attnv_half(3)
                    if USE_PAIR_DEN:
                        # pair denominator: rcp rows 64/65, one broadcast matmul
                        den2 = sb.tile([66, 512], bf, tag="rcp", bufs=2, name="den2")
                        with nc.allow_low_precision(reason="bf16 softmax denom"):
                            nc.vector.reciprocal(den2[64:65, :], po2[0][64:65, :])
                            nc.vector.reciprocal(den2[65:66, :], po2[1][65:66, :])
                        prb = psum()
                        nc.tensor.matmul(prb[:], selp[64:66, :], den2[64:66, :],
                                         start=True, stop=True)
                        for sub in range(2):
                            r0 = sub * 64
                            o16 = sb.tile([64, 512], f8, tag="o16", bufs=2, name="o16")
                            with nc.allow_low_precision(reason="fp8 attn output"):
                                nc.vector.tensor_tensor(o16[:], po2[sub][0:64, :],
                                                        prb[r0:r0 + 64, :], ALU.mult)
                            nc.sync.dma_start(oT_all[r0:r0 + 64, jo, :], o16[:])
                    else:
                        for sub in range(2):
                            r0 = sub * 64
                            po = po2[sub]
                            rcp = sb.tile([65, 512], bf, tag="rcp", bufs=2, name="rcp")
                            with nc.allow_low_precision(reason="bf16 softmax denom"):
                                nc.vector.reciprocal(rcp[64:65, :], po[64:65, :])
                            prb = psum()
                            nc.tensor.matmul(prb[0:64, :], ones_b[64:65, 0:64],
                                             rcp[64:65, :], start=True, stop=True)
                            rb = sb.tile([64, 512], bf, tag="rb", bufs=2, name="rb")
                            nc.vector.tensor_copy(rb[:], prb[0:64, :])
                            o16 = sb.tile([64, 512], f8, tag="o16", bufs=2, name="o16")
                            with nc.allow_low_precision(reason="fp8 attn output"):
                                nc.vector.tensor_tensor(o16[:], po[0:64, :],
                                                        rb[:], ALU.mult)
                            nc.sync.dma_start(oT_all[r0:r0 + 64, jo, :], o16[:])

                # ---- P4: attn out (fp8 DR) + gated residual (bf16), fused
                # with LN2 stats so DVE/PE overlap per do-chunk ----
                x2 = []
                p12 = psum2()
                p1 = p12[:, 0:512]
                p2 = p12[:, 512:1024]
                for do in range(KO):
                    py = psum()
                    for kp in range(KO // 2):
                        nc.tensor.matmul(py[:],
                                         wo_sb[do // 4][:, 2 * kp:2 * kp + 2, ts(do % 4, 128)],
                                         oT_all[:, 2 * kp:2 * kp + 2, :],
                                         start=(kp == 0), stop=(kp == KO // 2 - 1),
                                         perf_mode=DR)
                    t = sb.tile([128, Q], bf, tag="qslab", bufs=5, name="tao")
                    nc.scalar.activation(t[:], py[:], AF.Identity,
                                         bias=bo_s[:, do:do + 1],
                                         scale=gm_s[:, do:do + 1])
                    xx = sb.tile([128, Q], bf, tag="act2k", bufs=10, name="x2")
                    nc.vector.tensor_tensor(xx[:], t[:], xb8[do][:, 0:Q], ALU.add)
                    x2.append(xx)
                    sq2 = sb.tile([128, Q], bf, tag="qslab", bufs=5, name="sq2")
                    nc.vector.tensor_tensor(sq2[:], xx[:], xx[:], ALU.mult)
                    nc.tensor.matmul(p1, ones_b[:], xx[:], start=(do == 0),
                                     stop=(do == KO - 1))
                    nc.tensor.matmul(p2, ones_b[:], sq2[:], start=(do == 0),
                                     stop=(do == KO - 1))

                # ---- P5: LN2 tail ----
                mu16 = sb.tile([128, 512], bf, tag="stats16", bufs=3, name="mu16b")
                with nc.allow_low_precision(reason="bf16 LN mean"):
                    nc.vector.tensor_scalar_mul(mu16[:], p1, 1.0 / D)
                ex2 = tmpf()
                nc.vector.tensor_scalar_mul(ex2[:], p2, 1.0 / D)
                var = tmpf()
                nc.vector.tensor_tensor(var[:], mu16[:], mu16[:], ALU.mult)
                nc.vector.tensor_tensor(var[:], ex2[:], var[:], ALU.subtract)
                sd2 = tmpf()
                nc.scalar.activation(sd2[:], var[:], AF.Sqrt, bias=eps_ap[:])
                rstd16 = sb.tile([128, 512], bf, tag="stats16", bufs=3, name="rstd16b")
                with nc.allow_low_precision(reason="bf16 LN rstd"):
                    nc.vector.reciprocal(rstd16[:], sd2[:])
                g2 = []
                for ko in range(KO):
                    tm2 = sb.tile([128, 512], bf, tag="qslab", bufs=5, name="tm2")
                    nc.vector.tensor_tensor(tm2[:], x2[ko][:], mu16[:], ALU.subtract)
                    gk = sb.tile([128, Q], bf, tag="act1k", bufs=10, name="g2")
                    nc.vector.tensor_tensor(gk[:], tm2[:], rstd16[:], ALU.mult)
                    g2.append(gk)

                # ---- P6/P7: MLP (bf16) ----
                w1_sb = w1_early + [wpiece(w1_d, i, [128, KO, 512], nsplit=4)
                                    for i in range(2, 8)]
                w2_sb = [wpiece(w2_d, i, [128, 32, 128], nsplit=4)
                         for i in range(8)]
                m16 = sb.tile([128, 32, Q], bf, tag="m16v", bufs=1, name="m16")
                for mo in range(32):
                    pm = psum()
                    for ko in range(KO):
                        nc.tensor.matmul(pm[:], w1_sb[mo // 4][:, ko, ts(mo % 4, 128)],
                                         g2[ko][:], start=(ko == 0),
                                         stop=(ko == KO - 1))
                    nc.scalar.activation(m16[:, mo], pm[:], AF.Gelu_apprx_tanh,
                                         bias=b1_s[:, mo:mo + 1], scale=1.0)
                yt_r = yt_d.rearrange("(ko p) t -> p ko t", p=128)
                for do in range(KO):
                    pz = psum()
                    for ko in range(32):
                        nc.tensor.matmul(pz[:], w2_sb[do][:, ko, :],
                                         m16[:, ko, :], start=(ko == 0), stop=(ko == 31))
                    t = tmpf()
                    nc.scalar.activation(t[:], pz[:], AF.Identity,
                                         bias=b2_s[:, do:do + 1],
                                         scale=gp_s[:, do:do + 1])
                    yk = sb.tile([128, Q], f32, tag="yout", bufs=4, name="yout")
                    nc.vector.tensor_tensor(yk[:], t[:], x2[do][:], ALU.add)
                    for tb in range(2):
                        nc.sync.dma_start(yt_r[:, do, ts(tb, 256)],
                                          yk[:, ts(tb, 256)])

    nc.compile()
    return nc


# ----------------------------------------------------------------------------
# host wrapper
# ----------------------------------------------------------------------------

def _prep_shared(inputs):
    x = np.asarray(inputs["x"], np.float32)
    c = np.asarray(inputs["c"], np.float32)
    w_ada = np.asarray(inputs["w_ada"], np.float32)
    b_ada = np.asarray(inputs["b_ada"], np.float32)
    w_qkv = np.asarray(inputs["w_qkv"], np.float32)
    w_ao = np.asarray(inputs["w_attn_out"], np.float32)
    w_m1 = np.asarray(inputs["w_mlp1"], np.float32)
    w_m2 = np.asarray(inputs["w_mlp2"], np.float32)

    mod = c @ w_ada + b_ada
    sh_msa, sc_msa, g_msa, sh_mlp, sc_mlp, g_mlp = np.split(mod, 6, axis=1)
    ln1 = np.asarray(inputs["w_ln1"], np.float32) * (1.0 + sc_msa)   # [4, D]
    ln2 = np.asarray(inputs["w_ln2"], np.float32) * (1.0 + sc_mlp)

    shared = {}
    for b in range(B):
        Wq = w_qkv[:, :D] * ln1[b][:, None]
        Wk = w_qkv[:, D:2 * D] * ln1[b][:, None]
        Wv = w_qkv[:, 2 * D:] * ln1[b][:, None]
        bqkv = sh_msa[b] @ w_qkv
        W1 = w_m1 * ln2[b][:, None]
        bm1 = sh_mlp[b] @ w_m1 + np.asarray(inputs["b_mlp1"], np.float32)
        bvec = np.concatenate([
            _pvec(bqkv[:D]), _pvec(bqkv[D:2 * D]),
            _pvec((bqkv[2 * D:] @ w_ao) * g_msa[b]),
            _pvec(g_msa[b] / SW), _pvec(bm1),
            _pvec(np.asarray(inputs["b_mlp2"], np.float32) * g_mlp[b]),
            _pvec(g_mlp[b]),
        ], axis=1)
        shared[b] = dict(
            wq=_pieces(Wq, 512, F8, SW), wk=_pieces(Wk, 512, F8, SW),
            wv=_pieces(Wv, 512, F8, SW),
            wm1=_pieces(W1, 512),
            bvec=np.ascontiguousarray(bvec),
        )
    wao_p = _pieces(w_ao, 512, F8, SW)
    wm2_p = _pieces(w_m2, 128)
    bm2_p = _pvec(np.asarray(inputs["b_mlp2"], np.float32))
    cos = np.asarray(inputs["cos"], np.float32)
    sin = np.asarray(inputs["sin"], np.float32)
    return shared, wao_p, wm2_p, bm2_p, x, cos, sin


def _make_in_maps(inputs):
    shared, wao_p, wm2_p, bm2_p, x, cos, sin = _prep_shared(inputs)
    in_maps = []
    for core in range(8):
        b, half = core // 2, core % 2
        qlo = half * Q
        order = np.concatenate([np.arange(qlo, qlo + Q), np.arange(0, qlo),
                                np.arange(qlo + Q, S)])
        xT = x[b][order].T
        cosT = cos[order].T                      # [32, S]
        sinT = sin[order].T
        cc = np.concatenate([cosT] * 4, 0).astype(BF)
        ss = np.concatenate([-sinT, sinT, -sinT, sinT], 0).astype(BF)
        sh = shared[b]
        in_maps.append({
            "xb": np.ascontiguousarray(xT.astype(BF)),
            "wq": sh["wq"], "wk": sh["wk"], "wv": sh["wv"],
            "wao": wao_p, "wm1": sh["wm1"], "wm2": wm2_p,
            "cc": np.ascontiguousarray(cc), "ss": np.ascontiguousarray(ss),
            "bvec": sh["bvec"],
        })
    return in_maps


def kernel(**inputs):
    from concourse import bass_utils

    if "nc" not in _CACHE:
        _CACHE["nc"] = _build_program()
    nc = _CACHE["nc"]

    in_maps = _make_in_maps(inputs)
    res = bass_utils.run_bass_kernel_spmd(nc, in_maps, core_ids=list(range(8)))

    y = np.zeros((B, S, D), np.float32)
    for core in range(8):
        b, half = core // 2, core % 2
        qlo = half * Q
        y[b, qlo:qlo + Q] = res.results[core]["yt"].T
    return y
